# revision 27
# baseline (speedup 1.0000x reference)
"""OFA attention (dense_transformer) on 8 Trainium2 NeuronCores.

Sharding: heads split over cores (core c owns heads {2c, 2c+1}, both batches).
Per-core Bass/Tile program (see build_attention_nc below):
  phase 1 : QT/KT/VT = W_c @ hs.T (transposed projections; SCALING folded into Wq,
            c_attn folded into Wv on host; hsT DMA'd in 512-col chunks so the first
            matmul starts ~4us in; PSUM drained on ScalarE with fused bias-add
            while its exp stream hasn't started yet)
  phase 1b: V natural = PE-transpose(VT), packed [V_A | 1 | V_B | 1] bf16
  phase 2 : per (batch, 512-token t-block), streaming 128-row s-tiles:
              ST(s,t) = K Q^T           (M-split 64x64 PE tiles T0/T2/T8/T10; the
                                         two tiles of a column-pair run concurrently;
                                         per-(pair,head) PSUM tiles double-buffer so
                                         scores never serialize behind the exp)
              E = exp(ST) * expbT       (ScalarE exp PSUM -> SBUF bf16 per head;
                                         the multiply with host-precomputed
                                         exp(bias+mask) -- transposed to [s,t],
                                         bf16 -- alternates DVE/GpSimd:
                                         exp(s+b) == exp(s)*exp(b), so the bias
                                         never touches the PE and its DMA is halved)
              [O.T ; sums] += [V|1].T@E (PV matmul also produces softmax denoms;
                                         PV groups pop two-at-a-time AHEAD of the
                                         next scores to keep the PE fed)
            sums rows through DRAM into column orientation; one wide reciprocal;
            out-projection runs as M-split 64x64 tiles per head at the next
            t-block boundary; the 1/sums normalization applies at PSUM drain
            (both passes on DVE, ScalarE keeps only exp) and sums the heads.
Host: partial outputs summed over cores + bo (the "all-reduce" of the out-projection).
"""
import sys

for _p in ("/opt/trn_rl_repo",):
    if _p not in sys.path:
        sys.path.append(_p)

import numpy as np

import concourse.bass as bass
import concourse.tile as tile
from concourse import mybir
from concourse.masks import make_identity
from concourse.bass_utils import run_bass_kernel_spmd

F32 = mybir.dt.float32
BF16 = mybir.dt.bfloat16

B, T, E, NH, D = 2, 2048, 1024, 16, 64
N_CORES = 8
HPC = NH // N_CORES
DH = HPC * D
SCALING = float(D * 2.0) ** -0.5


def _waitfix(nc, limit=1):
    """This walrus build accepts at most ONE sync-wait per instruction.
    Hoist excess sem-waits onto inserted single-wait NoOps."""
    n_fixed = 0
    for bb in nc.m.functions[0].blocks:
        i = 0
        insts = bb.instructions
        while i < len(insts):
            inst = insts[i]
            si = inst.sync_info
            if si and si.on_wait and len(si.on_wait) > limit:
                extra = si.on_wait[limit:]
                si.on_wait = si.on_wait[:limit]
                for k, w in enumerate(extra):
                    nop = mybir.InstNoOp(
                        name=f"{inst.name}-waitfix{k}",
                        engine=inst.engine,
                        sync_info=mybir.SyncInfo(on_wait=[w], on_update=[]),
                        bass_nofuse=True,
                    )
                    nc.register_instruction(nop, overwrite=True)
                    insts.insert(i, nop)
                    i += 1
                n_fixed += 1
            i += 1
    return n_fixed


def build_attention_nc(B=2, T=2048, E=1024, HPC=2, D=64,
                       T_BLOCK=512, PROJ_BLOCK=512):
    """Build the per-core Bass program. Returns nc."""
    S = T
    PROJ_BLOCK = min(PROJ_BLOCK, T)
    TOK = B * T
    DH = HPC * D                      # 128
    assert DH == 128 and D == 64
    NE = E // 128                     # e-tiles
    NST = S // 128                    # s-tiles per batch
    NTB = T // T_BLOCK                # t-blocks per batch
    NJ = T_BLOCK // 128               # t-subtiles per block
    NPB = TOK // PROJ_BLOCK           # proj token blocks
    assert PROJ_BLOCK == 512

    nc = bass.Bass()

    hsT = nc.declare_dram_parameter("hsT", [E, TOK], BF16, isOutput=False)
    wqT = nc.declare_dram_parameter("wqT", [E, DH], BF16, isOutput=False)
    wkT = nc.declare_dram_parameter("wkT", [E, DH], BF16, isOutput=False)
    wvT = nc.declare_dram_parameter("wvT", [E, DH], BF16, isOutput=False)
    bq = nc.declare_dram_parameter("bq", [DH, 1], F32, isOutput=False)
    bk = nc.declare_dram_parameter("bk", [DH, 1], F32, isOutput=False)
    bv = nc.declare_dram_parameter("bv", [DH, 1], F32, isOutput=False)
    woT = nc.declare_dram_parameter("woT", [DH, E], BF16, isOutput=False)
    # exp(bias+mask) transposed to [s, t], bf16
    bias_in = nc.declare_dram_parameter("bias", [B, HPC, S, T], BF16,
                                        isOutput=False)
    out_partial = nc.declare_dram_parameter("out", [TOK, E], BF16, isOutput=True)
    rc_dram = nc.dram_tensor("rc_scratch", [B * NTB * HPC, T_BLOCK], F32)

    with tile.TileContext(nc) as tc:
        from contextlib import ExitStack
        with ExitStack() as ctx:
            consts = ctx.enter_context(tc.tile_pool(name="consts", bufs=1))
            persist = ctx.enter_context(tc.tile_pool(name="persist", bufs=1))
            expb_pool = ctx.enter_context(
                tc.tile_pool(name="expb_sb", bufs=12, space="SBUF"))

            i_bf = consts.tile([128, 128], BF16, tag="i_bf")
            make_identity(nc, i_bf[:])

            # weights: (E, DH) -> (128, NE, DH), bf16
            w_sb = {}
            for name, src in (("wq", wqT), ("wk", wkT), ("wv", wvT)):
                t = consts.tile([128, NE, DH], BF16, tag=name)
                nc.sync.dma_start(out=t[:], in_=src.rearrange("(n p) d -> p n d", p=128))
                w_sb[name] = t
            wo_sb = consts.tile([128, E], BF16, tag="wo")
            nc.sync.dma_start(out=wo_sb[:], in_=woT[:, :])
            b_sb = {}
            for name, src in (("bq", bq), ("bk", bk), ("bv", bv)):
                t = consts.tile([128, 1], F32, tag=name)
                nc.sync.dma_start(out=t[:], in_=src[:, :])
                b_sb[name] = t

            # persistent activations (QT/KT bf16; VT f32 for the PE transpose)
            QTb = [persist.tile([128, T], BF16, tag=f"QT{bb}", name=f"QT{bb}")
                   for bb in range(B)]
            KTb = [persist.tile([128, T], BF16, tag=f"KT{bb}", name=f"KT{bb}")
                   for bb in range(B)]
            VTb = [persist.tile([128, T], BF16, tag=f"VT{bb}", name=f"VT{bb}")
                   for bb in range(B)]
            V_sbb = []
            for bb in range(B):
                V_sb = persist.tile([128, T // 128, 256], BF16, tag=f"V_sb{bb}",
                                    name=f"V_sb{bb}")
                nc.vector.memset(V_sb[:, :, :], 0.0)
                nc.vector.memset(V_sb[:, :, D:D + 1], 1.0)
                nc.vector.memset(V_sb[:, :, 128 + D:128 + D + 1], 1.0)
                V_sbb.append(V_sb)

            # ---------------- phase 1: projections ----------------
            # hsT loaded in [128, 512] chunks, issued in consumption order so the
            # first matmul can start after ~1 MB instead of ~8 MB.
            with tc.tile_pool(name="hst", bufs=B * NE * (T // 512)) as hst_pool, \
                 tc.tile_pool(name="proj_ps", bufs=3, space="PSUM") as proj_ps:
                hstrips = {}
                for bb2 in range(B):
                    for c in range(T // 512):
                        for e in range(NE):
                            h = hst_pool.tile([128, 512], BF16, tag="hst",
                                              name=f"hst{bb2}_{e}_{c}")
                            nc.sync.dma_start(
                                out=h[:], in_=hsT[e * 128:(e + 1) * 128,
                                                  bb2 * T + c * 512:
                                                  bb2 * T + (c + 1) * 512])
                            hstrips[(bb2, e, c)] = h
                for pb in range(NPB):
                    t0 = pb * PROJ_BLOCK
                    bb = t0 // T
                    c = (t0 % T) // 512
                    tloc = t0 % T
                    for name, dstl in (("wq", QTb), ("wk", KTb), ("wv", VTb)):
                        ps = proj_ps.tile([128, PROJ_BLOCK], F32, tag="proj",
                                          name=f"pps{pb}_{name}")
                        for e in range(NE):
                            nc.tensor.matmul(ps[:], w_sb[name][:, e, :],
                                             hstrips[(bb, e, c)][:],
                                             start=(e == 0), stop=(e == NE - 1))
                        nc.scalar.activation(
                            out=dstl[bb][:, tloc:tloc + PROJ_BLOCK], in_=ps[:],
                            func=mybir.ActivationFunctionType.Identity,
                            bias=b_sb["b" + name[1]][:], scale=1.0)

            # ---------------- phase 1b: V natural ----------------
            with tc.tile_pool(name="vtr_ps", bufs=2, space="PSUM") as vtr_ps:
                for bb in range(B):
                    for st in range(T // 128):
                        ps = vtr_ps.tile([128, 128], BF16, tag="vtr",
                                         name=f"vtr{bb}_{st}")
                        nc.tensor.transpose(ps[:], VTb[bb][:, st * 128:(st + 1) * 128],
                                            i_bf[:])
                        nc.vector.tensor_copy(out=V_sbb[bb][:, st, 0:D],
                                              in_=ps[:, 0:D])
                        nc.vector.tensor_copy(out=V_sbb[bb][:, st, 128:128 + D],
                                              in_=ps[:, D:2 * D])

            # ---------------- phase 2: attention ----------------
            with tc.tile_pool(name="eraw_sb", bufs=3) as eraw_pool, \
                 tc.tile_pool(name="e_sb", bufs=5) as e_pool, \
                 tc.tile_pool(name="ot_sb", bufs=2) as ot_sb_pool, \
                 tc.tile_pool(name="sums", bufs=4) as sums_pool, \
                 tc.tile_pool(name="rcol", bufs=2) as rcol_pool, \
                 tc.tile_pool(name="tmp", bufs=3) as tmp_pool, \
                 tc.tile_pool(name="osb", bufs=3) as out_pool, \
                 tc.tile_pool(name="st_ps", bufs=1, space="PSUM") as st_ps, \
                 tc.tile_pool(name="ot_ps", bufs=2, space="PSUM") as ot_ps, \
                 tc.tile_pool(name="wo_ps", bufs=2, space="PSUM") as wo_ps:
                def make_wo_chunks(pw):
                    """Per-(k,n0) closures: the out-projection of block X runs
                    one chunk per sp iteration of block X+1, inside the PE
                    bubbles left by the scores->exp serialization, instead of
                    as a burst at the boundary that stalls the exp stream."""
                    otn_p, rcol_p, tglob_p = pw
                    os_tiles = {}

                    def chunk(k, n0):
                        if k in os_tiles:
                            os_t = os_tiles[k]
                        else:
                            os_t = os_tiles[k] = out_pool.tile(
                                [128, E], BF16, tag="osb",
                                name=f"osb{tglob_p}_{k}")
                        if True:
                            nn_ = min(512, E - n0)
                            wpa = wo_ps.tile([128, 512], F32, tag="wo",
                                             name=f"wopa{tglob_p}_{k}_{n0}")
                            wpb = wo_ps.tile([128, 512], F32, tag="wo",
                                             name=f"wopb{tglob_p}_{k}_{n0}")
                            # M-split 64x64 tiles: both heads x both t-halves
                            # run concurrently in the tiled PE array
                            for m0 in (0, 64):
                                nc.tensor.matmul(
                                    wpa[m0:m0 + 64, 0:nn_],
                                    otn_p[0:D, k * 128 + m0:k * 128 + m0 + 64],
                                    wo_sb[0:D, n0:n0 + nn_],
                                    start=True, stop=True)
                                nc.tensor.matmul(
                                    wpb[m0:m0 + 64, 0:nn_],
                                    otn_p[D:2 * D, k * 128 + m0:k * 128 + m0 + 64],
                                    wo_sb[D:2 * D, n0:n0 + nn_],
                                    start=True, stop=True)
                            tmp = tmp_pool.tile([128, 512], F32, tag="tmp",
                                                name=f"tmp{tglob_p}_{k}_{n0}")
                            # both normalization passes on DVE: nothing but exp
                            # may sit on the ScalarE queue between exps
                            nc.vector.tensor_scalar_mul(
                                out=tmp[:, 0:nn_], in0=wpa[:, 0:nn_],
                                scalar1=rcol_p[:, 0 * NJ + k:0 * NJ + k + 1])
                            nc.vector.scalar_tensor_tensor(
                                out=os_t[:, n0:n0 + nn_], in0=wpb[:, 0:nn_],
                                scalar=rcol_p[:, 1 * NJ + k:1 * NJ + k + 1],
                                in1=tmp[:, 0:nn_],
                                op0=mybir.AluOpType.mult,
                                op1=mybir.AluOpType.add)
                        if n0 + 512 >= E:
                            nc.gpsimd.dma_start(
                                out=out_partial[tglob_p + k * 128:
                                                tglob_p + (k + 1) * 128, :],
                                in_=os_t[:])

                    return [(chunk, k, n0) for k in range(NJ)
                            for n0 in range(0, E, 512)]

                pending_wo = []
                mult_flip = 0
                for b in range(B):
                    for tb in range(NTB):
                        tglob = b * T + tb * T_BLOCK
                        # exp(bias).T tiles: [128 s, (head, s-tile) plane, T t]
                        # per s-tile-pair, both heads in one tile (planes
                        # a*2+half), on the sync HWDGE queue: FIFO behind the
                        # hsT chunks, so phase-1 DMA wins the start
                        ebs = [None] * (NST // 2)
                        for sp in range(NST // 2):
                            t = expb_pool.tile([128, 4, T_BLOCK], BF16,
                                               tag="expb",
                                               name=f"expb{b}_{tb}_{sp}")
                            r0 = sp * 256
                            for a in range(HPC):
                                nc.sync.dma_start(
                                    out=t[:, 2 * a:2 * a + 2, :],
                                    in_=bias_in[b, a, r0:r0 + 256,
                                                tb * T_BLOCK:(tb + 1) * T_BLOCK]
                                    .rearrange("(k p) t -> p k t", p=128))
                            ebs[sp] = t

                        ots = [ot_ps.tile([128, T_BLOCK], F32, tag="ot",
                                          name=f"ot{b}_{tb}_{a}") for a in range(HPC)]

                        def emit_pv(pend):
                            for a, e_t, pst in pend:
                                nc.tensor.matmul(
                                    ots[a][:],
                                    V_sbb[b][:, pst, a * 128:a * 128 + 128],
                                    e_t[:],
                                    start=(pst == 0), stop=(pst == NST - 1))

                        pendq = []
                        for sp in range(NST // 2):
                            # PV ahead of scores keeps the PE fed while the
                            # scores' PSUM WAR on the previous exp resolves
                            if len(pendq) >= 3:
                                emit_pv(pendq.pop(0))
                                emit_pv(pendq.pop(0))
                            stp = st_ps.tile([128, 4, T_BLOCK], F32, tag="st",
                                             name=f"st{b}_{tb}_{sp}")
                            # M-split 64x64 tiles: (head a, out-half m0) -> PE
                            # tiles (r0, m0) = T0/T2/T8/T10
                            for half in range(2):
                                st = sp * 2 + half
                                for a in range(HPC):
                                    r0 = a * D
                                    for m0 in (0, 64):
                                        nc.tensor.matmul(
                                            stp[m0:m0 + 64, 2 * a + half, :],
                                            KTb[b][r0:r0 + D,
                                                   st * 128 + m0:st * 128 + m0 + 64],
                                            QTb[b][r0:r0 + D,
                                                   tb * T_BLOCK:tb * T_BLOCK + T_BLOCK],
                                            start=True, stop=True)
                            e_r = eraw_pool.tile([128, 4, T_BLOCK], BF16, tag="er",
                                                 name=f"er{b}_{tb}_{sp}")
                            nc.scalar.activation(
                                out=e_r[:], in_=stp[:],
                                func=mybir.ActivationFunctionType.Exp)
                            e_t = e_pool.tile([128, 4, T_BLOCK], BF16, tag="et",
                                              name=f"et{b}_{tb}_{sp}")
                            nc.vector.tensor_mul(out=e_t[:], in0=e_r[:],
                                                 in1=ebs[sp][:])
                            pend = []
                            for a in range(HPC):
                                for half in range(2):
                                    pend.append((a, e_t[:, 2 * a + half, :],
                                                 sp * 2 + half))
                            pendq.append(pend)
                            # one out-projection chunk of the previous t-block
                            if pending_wo:
                                fn, k_, n_ = pending_wo.pop(0)
                                fn(k_, n_)
                        for pend in pendq:
                            emit_pv(pend)

                        # drain O.T (bf16) + sums rows; frees ot psum quickly
                        otn = ot_sb_pool.tile([128, T_BLOCK], BF16, tag="otn",
                                              name=f"otn{b}_{tb}")
                        scol = sums_pool.tile([128, HPC * NJ], F32, tag="scol",
                                              name=f"scol{b}_{tb}")
                        for a in range(HPC):
                            nc.vector.tensor_copy(out=otn[a * D:(a + 1) * D, :],
                                                  in_=ots[a][0:D, :])
                            ss = sums_pool.tile([1, T_BLOCK], F32, tag="sums",
                                                name=f"sums{b}_{tb}_{a}")
                            nc.vector.tensor_copy(out=ss[:], in_=ots[a][D:D + 1, :])
                            idx = (b * NTB + tb) * HPC + a
                            nc.gpsimd.dma_start(out=rc_dram[idx, :], in_=ss[:])
                            nc.gpsimd.dma_start(
                                out=scol[:, a * NJ:(a + 1) * NJ],
                                in_=rc_dram[idx, :].rearrange("(k p) -> p k", p=128))
                        rcol = rcol_pool.tile([128, HPC * NJ], F32, tag="rcol",
                                              name=f"rcol{b}_{tb}")
                        nc.vector.reciprocal(rcol[:], scol[:])

                        for fn, k_, n_ in pending_wo:  # flush leftovers
                            fn(k_, n_)
                        pending_wo = make_wo_chunks((otn, rcol, tglob))
                for fn, k_, n_ in pending_wo:
                    fn(k_, n_)
    _waitfix(nc)
    return nc


# ---------------- host-side prep ----------------

def shard_inputs(hidden_states, attn_bias, attention_mask, Wq, bq, Wk, bk, Wv, bv,
                 Wo, bo, c_attn, n_cores=8, scaling=None):
    """Build per-core input maps. Returns (in_maps, with_mask)."""
    import ml_dtypes
    bf16 = ml_dtypes.bfloat16
    B, T, E = hidden_states.shape
    NH = c_attn.shape[0]
    D = E // NH
    HPC = NH // n_cores
    DH = HPC * D

    with_mask = bool(np.any(attention_mask))
    hsT = np.ascontiguousarray(hidden_states.reshape(B * T, E).T).astype(bf16)
    bias4 = attn_bias.reshape(B, NH, T, T)
    if with_mask:
        bias4 = bias4 + attention_mask.reshape(B, 1, T, T)

    # exp(bias+mask) as bf16 (viewed as uint16 so the per-core transpose below
    # takes numpy's fast strided-copy path)
    expb_u16 = np.exp(bias4).astype(bf16).view(np.uint16)

    if scaling is None:
        scaling = float(D * 2.0) ** -0.5

    in_maps = []
    for c in range(n_cores):
        r0 = c * DH
        sl = slice(r0, r0 + DH)
        hsl = slice(c * HPC, (c + 1) * HPC)
        cvec = np.repeat(c_attn[c * HPC:(c + 1) * HPC], D)
        m = {
            "hsT": hsT,
            "wqT": np.ascontiguousarray((Wq[sl] * scaling).T).astype(bf16),
            "wkT": np.ascontiguousarray(Wk[sl].T).astype(bf16),
            "wvT": np.ascontiguousarray((Wv[sl] * cvec[:, None]).T).astype(bf16),
            "bq": np.ascontiguousarray((bq[sl] * scaling)[:, None]).astype(np.float32),
            "bk": np.ascontiguousarray(bk[sl][:, None]).astype(np.float32),
            "bv": np.ascontiguousarray((bv[sl] * cvec)[:, None]).astype(np.float32),
            "woT": np.ascontiguousarray(Wo[:, sl].T).astype(bf16),
            # [B, HPC, S, T]: transposed exp-bias for this core's heads
            "bias": np.ascontiguousarray(
                expb_u16[:, hsl].transpose(0, 1, 3, 2)).view(bf16),
        }
        in_maps.append(m)
    return in_maps, with_mask


_NC_CACHE = {}


def run_spmd(in_maps, with_mask=False, **kwargs):
    if "nc" not in _NC_CACHE:
        _NC_CACHE["nc"] = build_attention_nc(B=B, T=T, E=E, HPC=HPC, D=D)
    nc = _NC_CACHE["nc"]
    return run_bass_kernel_spmd(nc, in_maps, list(range(N_CORES)), **kwargs)


def kernel(hidden_states, attn_bias, attention_mask, Wq, bq, Wk, bk, Wv, bv,
           Wo, bo, c_attn):
    args = [np.asarray(a, dtype=np.float32) for a in
            (hidden_states, attn_bias, attention_mask, Wq, bq, Wk, bk, Wv, bv,
             Wo, bo, c_attn)]
    (hidden_states, attn_bias, attention_mask, Wq, bq, Wk, bk, Wv, bv,
     Wo, bo, c_attn) = args
    in_maps, with_mask = shard_inputs(hidden_states, attn_bias, attention_mask,
                                      Wq, bq, Wk, bk, Wv, bv, Wo, bo, c_attn,
                                      n_cores=N_CORES, scaling=SCALING)
    res = run_spmd(in_maps, with_mask)
    out = np.zeros((B * T, E), np.float32)
    for r in res.results:
        out += r["out"]
    out += bo[None, :]
    return out.reshape(B, T, E).astype(np.float32)


# revision 28
# speedup vs baseline: 1.0695x; 1.0695x over previous
"""OFA attention (dense_transformer) on 8 Trainium2 NeuronCores.

Sharding: heads split over cores (core c owns heads {2c, 2c+1}, both batches).
Per-core Bass/Tile program (see build_attention_nc below):
  phase 1 : QT/KT/VT = W_c @ hs.T (transposed projections; SCALING folded into Wq,
            c_attn folded into Wv on host; hsT DMA'd in 512-col chunks so the first
            matmul starts ~4us in; PSUM drained on ScalarE with fused bias-add
            while its exp stream hasn't started yet)
  phase 1b: V natural = PE-transpose(VT), packed [V_A | 1 | V_B | 1] bf16
  phase 2 : per (batch, 512-token t-block), streaming 128-row s-tiles:
              ST(s,t) = K Q^T           (M-split 64x64 PE tiles T0/T2/T8/T10; the
                                         two tiles of a column-pair run concurrently;
                                         per-(pair,head) PSUM tiles double-buffer so
                                         scores never serialize behind the exp)
              E = exp(ST) * expbT       (ScalarE exp PSUM -> SBUF bf16 per head;
                                         the multiply with host-precomputed
                                         exp(bias+mask) -- transposed to [s,t],
                                         bf16 -- alternates DVE/GpSimd:
                                         exp(s+b) == exp(s)*exp(b), so the bias
                                         never touches the PE and its DMA is halved)
              [O.T ; sums] += [V|1].T@E (PV matmul also produces softmax denoms;
                                         PV groups pop two-at-a-time AHEAD of the
                                         next scores to keep the PE fed)
            sums rows through DRAM into column orientation; one wide reciprocal;
            out-projection runs as M-split 64x64 tiles per head at the next
            t-block boundary; the 1/sums normalization applies at PSUM drain
            (both passes on DVE, ScalarE keeps only exp) and sums the heads.
Host: partial outputs summed over cores + bo (the "all-reduce" of the out-projection).
"""
import sys

for _p in ("/opt/trn_rl_repo",):
    if _p not in sys.path:
        sys.path.append(_p)

import numpy as np

import concourse.bass as bass
import concourse.tile as tile
from concourse import mybir
from concourse.masks import make_identity
from concourse.bass_utils import run_bass_kernel_spmd

F32 = mybir.dt.float32
BF16 = mybir.dt.bfloat16

B, T, E, NH, D = 2, 2048, 1024, 16, 64
N_CORES = 8
HPC = NH // N_CORES
DH = HPC * D
SCALING = float(D * 2.0) ** -0.5


def _waitfix(nc, limit=1):
    """This walrus build accepts at most ONE sync-wait per instruction.
    Hoist excess sem-waits onto inserted single-wait NoOps."""
    n_fixed = 0
    for bb in nc.m.functions[0].blocks:
        i = 0
        insts = bb.instructions
        while i < len(insts):
            inst = insts[i]
            si = inst.sync_info
            if si and si.on_wait and len(si.on_wait) > limit:
                extra = si.on_wait[limit:]
                si.on_wait = si.on_wait[:limit]
                for k, w in enumerate(extra):
                    nop = mybir.InstNoOp(
                        name=f"{inst.name}-waitfix{k}",
                        engine=inst.engine,
                        sync_info=mybir.SyncInfo(on_wait=[w], on_update=[]),
                        bass_nofuse=True,
                    )
                    nc.register_instruction(nop, overwrite=True)
                    insts.insert(i, nop)
                    i += 1
                n_fixed += 1
            i += 1
    return n_fixed


def build_attention_nc(B=2, T=2048, E=1024, HPC=2, D=64,
                       T_BLOCK=512, PROJ_BLOCK=512):
    """Build the per-core Bass program. Returns nc."""
    S = T
    PROJ_BLOCK = min(PROJ_BLOCK, T)
    TOK = B * T
    DH = HPC * D                      # 128
    assert DH == 128 and D == 64
    NE = E // 128                     # e-tiles
    NST = S // 128                    # s-tiles per batch
    NTB = T // T_BLOCK                # t-blocks per batch
    NJ = T_BLOCK // 128               # t-subtiles per block
    NPB = TOK // PROJ_BLOCK           # proj token blocks
    assert PROJ_BLOCK == 512

    nc = bass.Bass()

    hsT = nc.declare_dram_parameter("hsT", [E, TOK], BF16, isOutput=False)
    wqT = nc.declare_dram_parameter("wqT", [E, DH], BF16, isOutput=False)
    wkT = nc.declare_dram_parameter("wkT", [E, DH], BF16, isOutput=False)
    wvT = nc.declare_dram_parameter("wvT", [E, DH], BF16, isOutput=False)
    bq = nc.declare_dram_parameter("bq", [DH, 1], F32, isOutput=False)
    bk = nc.declare_dram_parameter("bk", [DH, 1], F32, isOutput=False)
    bv = nc.declare_dram_parameter("bv", [DH, 1], F32, isOutput=False)
    woT = nc.declare_dram_parameter("woT", [DH, E], BF16, isOutput=False)
    # exp(bias+mask) transposed to [s, t], bf16
    bias_in = nc.declare_dram_parameter("bias", [B, HPC, S, T], BF16,
                                        isOutput=False)
    out_partial = nc.declare_dram_parameter("out", [TOK, E], BF16, isOutput=True)
    rc_dram = nc.dram_tensor("rc_scratch", [B * NTB * HPC, T_BLOCK], F32)

    with tile.TileContext(nc) as tc:
        from contextlib import ExitStack
        with ExitStack() as ctx:
            consts = ctx.enter_context(tc.tile_pool(name="consts", bufs=1))
            persist = ctx.enter_context(tc.tile_pool(name="persist", bufs=1))
            expb_pool = ctx.enter_context(
                tc.tile_pool(name="expb_sb", bufs=12, space="SBUF"))

            i_bf = consts.tile([128, 128], BF16, tag="i_bf")
            make_identity(nc, i_bf[:])

            # weights: (E, DH) -> (128, NE, DH), bf16
            w_sb = {}
            for name, src in (("wq", wqT), ("wk", wkT), ("wv", wvT)):
                t = consts.tile([128, NE, DH], BF16, tag=name)
                nc.sync.dma_start(out=t[:], in_=src.rearrange("(n p) d -> p n d", p=128))
                w_sb[name] = t
            wo_sb = consts.tile([128, E], BF16, tag="wo")
            nc.sync.dma_start(out=wo_sb[:], in_=woT[:, :])
            b_sb = {}
            for name, src in (("bq", bq), ("bk", bk), ("bv", bv)):
                t = consts.tile([128, 1], F32, tag=name)
                nc.sync.dma_start(out=t[:], in_=src[:, :])
                b_sb[name] = t

            # persistent activations (QT/KT bf16; VT f32 for the PE transpose)
            QTb = [persist.tile([128, T], BF16, tag=f"QT{bb}", name=f"QT{bb}")
                   for bb in range(B)]
            KTb = [persist.tile([128, T], BF16, tag=f"KT{bb}", name=f"KT{bb}")
                   for bb in range(B)]
            VTb = [persist.tile([128, T], BF16, tag=f"VT{bb}", name=f"VT{bb}")
                   for bb in range(B)]
            V_sbb = []
            for bb in range(B):
                V_sb = persist.tile([128, T // 128, 256], BF16, tag=f"V_sb{bb}",
                                    name=f"V_sb{bb}")
                nc.vector.memset(V_sb[:, :, :], 0.0)
                nc.vector.memset(V_sb[:, :, D:D + 1], 1.0)
                nc.vector.memset(V_sb[:, :, 128 + D:128 + D + 1], 1.0)
                V_sbb.append(V_sb)

            # ---------------- phase 1: projections ----------------
            # hsT loaded in [128, 512] chunks, issued in consumption order so the
            # first matmul can start after ~1 MB instead of ~8 MB.
            with tc.tile_pool(name="hst", bufs=B * NE * (T // 512)) as hst_pool, \
                 tc.tile_pool(name="proj_ps", bufs=3, space="PSUM") as proj_ps:
                hstrips = {}
                for bb2 in range(B):
                    for c in range(T // 512):
                        for e in range(NE):
                            h = hst_pool.tile([128, 512], BF16, tag="hst",
                                              name=f"hst{bb2}_{e}_{c}")
                            nc.sync.dma_start(
                                out=h[:], in_=hsT[e * 128:(e + 1) * 128,
                                                  bb2 * T + c * 512:
                                                  bb2 * T + (c + 1) * 512])
                            hstrips[(bb2, e, c)] = h
                for pb in range(NPB):
                    t0 = pb * PROJ_BLOCK
                    bb = t0 // T
                    c = (t0 % T) // 512
                    tloc = t0 % T
                    for name, dstl in (("wq", QTb), ("wk", KTb), ("wv", VTb)):
                        ps = proj_ps.tile([128, PROJ_BLOCK], F32, tag="proj",
                                          name=f"pps{pb}_{name}")
                        for e in range(NE):
                            nc.tensor.matmul(ps[:], w_sb[name][:, e, :],
                                             hstrips[(bb, e, c)][:],
                                             start=(e == 0), stop=(e == NE - 1))
                        nc.scalar.activation(
                            out=dstl[bb][:, tloc:tloc + PROJ_BLOCK], in_=ps[:],
                            func=mybir.ActivationFunctionType.Identity,
                            bias=b_sb["b" + name[1]][:], scale=1.0)

            # ---------------- phase 1b: V natural ----------------
            with tc.tile_pool(name="vtr_ps", bufs=2, space="PSUM") as vtr_ps:
                for bb in range(B):
                    for st in range(T // 128):
                        ps = vtr_ps.tile([128, 128], BF16, tag="vtr",
                                         name=f"vtr{bb}_{st}")
                        nc.tensor.transpose(ps[:], VTb[bb][:, st * 128:(st + 1) * 128],
                                            i_bf[:])
                        nc.vector.tensor_copy(out=V_sbb[bb][:, st, 0:D],
                                              in_=ps[:, 0:D])
                        nc.vector.tensor_copy(out=V_sbb[bb][:, st, 128:128 + D],
                                              in_=ps[:, D:2 * D])

            # ---------------- phase 2: attention ----------------
            with tc.tile_pool(name="eraw_sb", bufs=3) as eraw_pool, \
                 tc.tile_pool(name="e_sb", bufs=5) as e_pool, \
                 tc.tile_pool(name="ot_sb", bufs=2) as ot_sb_pool, \
                 tc.tile_pool(name="sums", bufs=4) as sums_pool, \
                 tc.tile_pool(name="rcol", bufs=2) as rcol_pool, \
                 tc.tile_pool(name="tmp", bufs=3) as tmp_pool, \
                 tc.tile_pool(name="osb", bufs=3) as out_pool, \
                 tc.tile_pool(name="st_ps", bufs=1, space="PSUM") as st_ps, \
                 tc.tile_pool(name="ot_ps", bufs=2, space="PSUM") as ot_ps, \
                 tc.tile_pool(name="wo_ps", bufs=2, space="PSUM") as wo_ps:
                def emit_wo(pw):
                    otn_p, rcol_p, tglob_p = pw
                    for k in range(NJ):
                        os_t = out_pool.tile([128, E], BF16, tag="osb",
                                             name=f"osb{tglob_p}_{k}")
                        for n0 in range(0, E, 512):
                            nn_ = min(512, E - n0)
                            wpa = wo_ps.tile([128, 512], F32, tag="wo",
                                             name=f"wopa{tglob_p}_{k}_{n0}")
                            wpb = wo_ps.tile([128, 512], F32, tag="wo",
                                             name=f"wopb{tglob_p}_{k}_{n0}")
                            # M-split 64x64 tiles: both heads x both t-halves
                            # run concurrently in the tiled PE array
                            for m0 in (0, 64):
                                nc.tensor.matmul(
                                    wpa[m0:m0 + 64, 0:nn_],
                                    otn_p[0:D, k * 128 + m0:k * 128 + m0 + 64],
                                    wo_sb[0:D, n0:n0 + nn_],
                                    start=True, stop=True)
                                nc.tensor.matmul(
                                    wpb[m0:m0 + 64, 0:nn_],
                                    otn_p[D:2 * D, k * 128 + m0:k * 128 + m0 + 64],
                                    wo_sb[D:2 * D, n0:n0 + nn_],
                                    start=True, stop=True)
                            tmp = tmp_pool.tile([128, 512], F32, tag="tmp",
                                                name=f"tmp{tglob_p}_{k}_{n0}")
                            nc.scalar.activation(
                                out=tmp[:, 0:nn_], in_=wpa[:, 0:nn_],
                                func=mybir.ActivationFunctionType.Copy,
                                scale=rcol_p[:, 0 * NJ + k:0 * NJ + k + 1])
                            # os = (wpb * rb) + tmp in one DVE op
                            nc.vector.scalar_tensor_tensor(
                                out=os_t[:, n0:n0 + nn_], in0=wpb[:, 0:nn_],
                                scalar=rcol_p[:, 1 * NJ + k:1 * NJ + k + 1],
                                in1=tmp[:, 0:nn_],
                                op0=mybir.AluOpType.mult,
                                op1=mybir.AluOpType.add)
                        nc.gpsimd.dma_start(
                            out=out_partial[tglob_p + k * 128: tglob_p + (k + 1) * 128, :],
                            in_=os_t[:])

                pending_wo = None
                mult_flip = 0
                for b in range(B):
                    for tb in range(NTB):
                        tglob = b * T + tb * T_BLOCK
                        # exp(bias).T tiles: [128 s, (head, s-tile) plane, T t]
                        # per s-tile-pair, both heads in one tile (planes
                        # a*2+half), on the sync HWDGE queue: FIFO behind the
                        # hsT chunks, so phase-1 DMA wins the start
                        ebs = [None] * (NST // 2)
                        for sp in range(NST // 2):
                            t = expb_pool.tile([128, 4, T_BLOCK], BF16,
                                               tag="expb",
                                               name=f"expb{b}_{tb}_{sp}")
                            r0 = sp * 256
                            for a in range(HPC):
                                nc.sync.dma_start(
                                    out=t[:, 2 * a:2 * a + 2, :],
                                    in_=bias_in[b, a, r0:r0 + 256,
                                                tb * T_BLOCK:(tb + 1) * T_BLOCK]
                                    .rearrange("(k p) t -> p k t", p=128))
                            ebs[sp] = t

                        ots = [ot_ps.tile([128, T_BLOCK], F32, tag="ot",
                                          name=f"ot{b}_{tb}_{a}") for a in range(HPC)]

                        def emit_pv(pend):
                            for a, e_t, pst in pend:
                                nc.tensor.matmul(
                                    ots[a][:],
                                    V_sbb[b][:, pst, a * 128:a * 128 + 128],
                                    e_t[:],
                                    start=(pst == 0), stop=(pst == NST - 1))

                        pendq = []
                        for sp in range(NST // 2):
                            # PV ahead of scores keeps the PE fed while the
                            # scores' PSUM WAR on the previous exp resolves
                            if len(pendq) >= 3:
                                emit_pv(pendq.pop(0))
                                emit_pv(pendq.pop(0))
                            stp = st_ps.tile([128, 4, T_BLOCK], F32, tag="st",
                                             name=f"st{b}_{tb}_{sp}")
                            # M-split 64x64 tiles: (head a, out-half m0) -> PE
                            # tiles (r0, m0) = T0/T2/T8/T10
                            for half in range(2):
                                st = sp * 2 + half
                                for a in range(HPC):
                                    r0 = a * D
                                    for m0 in (0, 64):
                                        nc.tensor.matmul(
                                            stp[m0:m0 + 64, 2 * a + half, :],
                                            KTb[b][r0:r0 + D,
                                                   st * 128 + m0:st * 128 + m0 + 64],
                                            QTb[b][r0:r0 + D,
                                                   tb * T_BLOCK:tb * T_BLOCK + T_BLOCK],
                                            start=True, stop=True)
                            e_r = eraw_pool.tile([128, 4, T_BLOCK], BF16, tag="er",
                                                 name=f"er{b}_{tb}_{sp}")
                            nc.scalar.activation(
                                out=e_r[:], in_=stp[:],
                                func=mybir.ActivationFunctionType.Exp)
                            e_t = e_pool.tile([128, 4, T_BLOCK], BF16, tag="et",
                                              name=f"et{b}_{tb}_{sp}")
                            nc.vector.tensor_mul(out=e_t[:], in0=e_r[:],
                                                 in1=ebs[sp][:])
                            pend = []
                            for a in range(HPC):
                                for half in range(2):
                                    pend.append((a, e_t[:, 2 * a + half, :],
                                                 sp * 2 + half))
                            pendq.append(pend)
                        for pend in pendq:
                            emit_pv(pend)

                        # drain O.T (bf16) + sums rows; frees ot psum quickly
                        otn = ot_sb_pool.tile([128, T_BLOCK], BF16, tag="otn",
                                              name=f"otn{b}_{tb}")
                        scol = sums_pool.tile([128, HPC * NJ], F32, tag="scol",
                                              name=f"scol{b}_{tb}")
                        for a in range(HPC):
                            nc.vector.tensor_copy(out=otn[a * D:(a + 1) * D, :],
                                                  in_=ots[a][0:D, :])
                            ss = sums_pool.tile([1, T_BLOCK], F32, tag="sums",
                                                name=f"sums{b}_{tb}_{a}")
                            nc.vector.tensor_copy(out=ss[:], in_=ots[a][D:D + 1, :])
                            idx = (b * NTB + tb) * HPC + a
                            nc.gpsimd.dma_start(out=rc_dram[idx, :], in_=ss[:])
                            nc.gpsimd.dma_start(
                                out=scol[:, a * NJ:(a + 1) * NJ],
                                in_=rc_dram[idx, :].rearrange("(k p) -> p k", p=128))
                        rcol = rcol_pool.tile([128, HPC * NJ], F32, tag="rcol",
                                              name=f"rcol{b}_{tb}")
                        nc.vector.reciprocal(rcol[:], scol[:])

                        if pending_wo is not None:
                            emit_wo(pending_wo)
                        pending_wo = (otn, rcol, tglob)
                emit_wo(pending_wo)
    _waitfix(nc)
    return nc


# ---------------- host-side prep ----------------

def shard_inputs(hidden_states, attn_bias, attention_mask, Wq, bq, Wk, bk, Wv, bv,
                 Wo, bo, c_attn, n_cores=8, scaling=None):
    """Build per-core input maps. Returns (in_maps, with_mask)."""
    import ml_dtypes
    bf16 = ml_dtypes.bfloat16
    B, T, E = hidden_states.shape
    NH = c_attn.shape[0]
    D = E // NH
    HPC = NH // n_cores
    DH = HPC * D

    with_mask = bool(np.any(attention_mask))
    hsT = np.ascontiguousarray(hidden_states.reshape(B * T, E).T).astype(bf16)
    bias4 = attn_bias.reshape(B, NH, T, T)
    if with_mask:
        bias4 = bias4 + attention_mask.reshape(B, 1, T, T)

    # exp(bias+mask) as bf16 (viewed as uint16 so the per-core transpose below
    # takes numpy's fast strided-copy path)
    expb_u16 = np.exp(bias4).astype(bf16).view(np.uint16)

    if scaling is None:
        scaling = float(D * 2.0) ** -0.5

    in_maps = []
    for c in range(n_cores):
        r0 = c * DH
        sl = slice(r0, r0 + DH)
        hsl = slice(c * HPC, (c + 1) * HPC)
        cvec = np.repeat(c_attn[c * HPC:(c + 1) * HPC], D)
        m = {
            "hsT": hsT,
            "wqT": np.ascontiguousarray((Wq[sl] * scaling).T).astype(bf16),
            "wkT": np.ascontiguousarray(Wk[sl].T).astype(bf16),
            "wvT": np.ascontiguousarray((Wv[sl] * cvec[:, None]).T).astype(bf16),
            "bq": np.ascontiguousarray((bq[sl] * scaling)[:, None]).astype(np.float32),
            "bk": np.ascontiguousarray(bk[sl][:, None]).astype(np.float32),
            "bv": np.ascontiguousarray((bv[sl] * cvec)[:, None]).astype(np.float32),
            "woT": np.ascontiguousarray(Wo[:, sl].T).astype(bf16),
            # [B, HPC, S, T]: transposed exp-bias for this core's heads
            "bias": np.ascontiguousarray(
                expb_u16[:, hsl].transpose(0, 1, 3, 2)).view(bf16),
        }
        in_maps.append(m)
    return in_maps, with_mask


_NC_CACHE = {}


def run_spmd(in_maps, with_mask=False, **kwargs):
    if "nc" not in _NC_CACHE:
        _NC_CACHE["nc"] = build_attention_nc(B=B, T=T, E=E, HPC=HPC, D=D)
    nc = _NC_CACHE["nc"]
    return run_bass_kernel_spmd(nc, in_maps, list(range(N_CORES)), **kwargs)


def kernel(hidden_states, attn_bias, attention_mask, Wq, bq, Wk, bk, Wv, bv,
           Wo, bo, c_attn):
    args = [np.asarray(a, dtype=np.float32) for a in
            (hidden_states, attn_bias, attention_mask, Wq, bq, Wk, bk, Wv, bv,
             Wo, bo, c_attn)]
    (hidden_states, attn_bias, attention_mask, Wq, bq, Wk, bk, Wv, bv,
     Wo, bo, c_attn) = args
    in_maps, with_mask = shard_inputs(hidden_states, attn_bias, attention_mask,
                                      Wq, bq, Wk, bk, Wv, bv, Wo, bo, c_attn,
                                      n_cores=N_CORES, scaling=SCALING)
    res = run_spmd(in_maps, with_mask)
    out = np.zeros((B * T, E), np.float32)
    for r in res.results:
        out += r["out"]
    out += bo[None, :]
    return out.reshape(B, T, E).astype(np.float32)
